# revision 7
# baseline (speedup 1.0000x reference)
"""DSAttention TRN2 Bass kernel.

Reference (per batch b, head h, branch):
    z[l,s] = (q[l]·k[s]) * tau[b]/8 + delta[b,s]/8        (causal: s <= l)
    A = softmax_s(z);  O = A @ V
    out = m*O_edit + (1-m)*O_null,  m = soft_mask[b,l]

Sharding: B*H = 16 (b,h) slices -> 8 cores x 2 heads. Same SPMD program on
every core; core c gets b = c//4, heads 2*(c%4), 2*(c%4)+1.

Per-core algorithm (transposed-score flash attention), v2 = bf16 + merged
tiles to unload the DVE (modeled baseline bottleneck ~92us busy):
  - Host pre-packs per head in bf16: qt = [Q^T; Q^T] (dup) and
    kt = [K_e^T; K_n^T] [128, L] so the two branches' QK^T matmuls run
    row-packed (tile_position (0,0)/(64,0)) concurrently on the PE array,
    and V with a ones column appended ([S, 65]) in natural layout.
  - scores^T tile [s:128, l:<=512] per (S-tile, L-chunk), trimmed to the
    causal region; exp on ACT with fused scale=tau/8 and bias=delta_s/8
    (per-partition APs), both branches in one ACTIVATE, bf16 out; diagonal
    128x128 block masked post-exp with ONE DVE multiply over the
    [128, 2, 128] branch-merged region (bf16 2x mode).
  - O^T accumulates AV matmuls (bf16) into a single merged PSUM tile
    [65, 2, 512] (row 64 of each branch half = softmax denominator via the
    ones column); ONE PSUM->SBUF copy per L-chunk.
  - epilogue per 128-l block: PE-transposes both branches into one PSUM
    bank [128, 2, 65]; DVE: reciprocal of the two denominators in one op,
    one multiply with the host-interleaved (m, 1-m) tile, then
    ts_mul + scalar_tensor_tensor blend -> [128, 64] f32 out.
  - PSUM banks: pt 2x2 + oac 2 + tr 2x1 = 8 (fully used).

REPEAT > 1 wraps the whole per-core program in a hardware For_i loop; used
by the timing harness to measure per-iteration HW time from wall-clock
deltas (transfers cancel).
"""

import contextlib

import ml_dtypes
import numpy as np

import concourse.bass as bass
import concourse.tile as tile
from concourse import bacc, mybir
from concourse.bass_utils import run_bass_kernel_spmd

B, L, S, H, E, D = 2, 2048, 2048, 8, 64, 64
NCORES = 8
HPC = 2            # heads per core
NT = 16            # 128-row tiles in 2048
LCH = 4            # 512-wide L chunks
F32 = mybir.dt.float32
BF16 = mybir.dt.bfloat16
EXPF = mybir.ActivationFunctionType.Exp
MUL = mybir.AluOpType.mult
ADD = mybir.AluOpType.add
BF16NP = ml_dtypes.bfloat16

TRACE = False
LAST_EXEC_NS = None
PTS_BUFS = 6
OSB_BUFS = 3
OB_BUFS = 4
REPEAT = 1

_NC = None


def _build():
    nc = bacc.Bacc("TRN2")
    qt_p = nc.declare_dram_parameter("qt", [HPC, 128, L], BF16, isOutput=False)
    kt_p = nc.declare_dram_parameter("kt", [HPC, 128, S], BF16, isOutput=False)
    v_p = nc.declare_dram_parameter("v", [HPC, S, D + 1], BF16, isOutput=False)
    vn_p = nc.declare_dram_parameter("vn", [HPC, S, D + 1], BF16, isOutput=False)
    st_p = nc.declare_dram_parameter("st", [128, 1], F32, isOutput=False)
    cd_p = nc.declare_dram_parameter("cdelta", [128, NT], F32, isOutput=False)
    mtb_p = nc.declare_dram_parameter("mtb", [128, NT, 2], F32, isOutput=False)
    id_p = nc.declare_dram_parameter("ident", [128, 128], BF16, isOutput=False)
    mk_p = nc.declare_dram_parameter("mask", [128, 2, 128], BF16, isOutput=False)
    out_p = nc.declare_dram_parameter("out", [HPC, L, D], F32, isOutput=True)
    params = (qt_p, kt_p, v_p, vn_p, st_p, cd_p, mtb_p, id_p, mk_p, out_p)

    with tile.TileContext(nc) as tc:
        with (
            tc.tile_pool(name="const", bufs=1) as const,
            tc.tile_pool(name="big", bufs=2) as big,
            tc.tile_pool(name="pts", bufs=PTS_BUFS) as pts,
            tc.tile_pool(name="osb", bufs=OSB_BUFS) as osb,
            tc.tile_pool(name="sml", bufs=8) as sml,
            tc.tile_pool(name="ob", bufs=OB_BUFS) as ob,
            tc.tile_pool(name="ps_pt", bufs=2, space="PSUM") as ps_pt,
            tc.tile_pool(name="ps_oac", bufs=1, space="PSUM") as ps_oac,
            tc.tile_pool(name="ps_tr", bufs=2, space="PSUM") as ps_tr,
        ):
            pools = (const, big, pts, osb, sml, ob, ps_pt, ps_oac, ps_tr)
            rep = (
                tc.For_i(0, REPEAT, 1)
                if REPEAT > 1
                else contextlib.nullcontext()
            )
            with rep:
                _body(nc, pools, params)
    if not nc.is_finalized():
        nc.finalize()
    return nc


def _body(nc, pools, params):
    const, big, pts, osb, sml, ob, ps_pt, ps_oac, ps_tr = pools
    qt_p, kt_p, v_p, vn_p, st_p, cd_p, mtb_p, id_p, mk_p, out_p = params

    ident = const.tile([128, 128], BF16, tag="ident")
    nc.sync.dma_start(out=ident, in_=id_p[:])
    mask = const.tile([128, 2, 128], BF16, tag="mask")
    nc.sync.dma_start(out=mask, in_=mk_p[:])
    cdelta = const.tile([128, NT], F32, tag="cdelta")
    nc.sync.dma_start(out=cdelta, in_=cd_p[:])
    mtb = const.tile([128, NT, 2], F32, tag="mtb")
    nc.sync.dma_start(out=mtb, in_=mtb_p[:])
    st = const.tile([128, 1], F32, tag="st")
    nc.sync.dma_start(out=st, in_=st_p[:])

    for bh in range(HPC):
        qt = big.tile([128, L], BF16, tag="qt")
        kt = big.tile([128, S], BF16, tag="kt")
        nc.sync.dma_start(out=qt, in_=qt_p[bh])
        nc.sync.dma_start(out=kt, in_=kt_p[bh])
        ve = big.tile([128, NT, D + 1], BF16, tag="ve")
        nc.sync.dma_start(
            out=ve, in_=v_p[bh].rearrange("(t p) d -> p t d", p=128)
        )
        vn = big.tile([128, NT, D + 1], BF16, tag="vn")
        nc.sync.dma_start(
            out=vn, in_=vn_p[bh].rearrange("(t p) d -> p t d", p=128)
        )
        for lc in range(LCH):
            lcb = 512 * lc
            oac = ps_oac.tile([D + 1, 2, 512], F32, tag="oac")
            njs = 4 * lc + 4
            for js in range(njs):
                off = max(0, 128 * js - lcb)
                sb = 128 * js
                lsl = slice(lcb + off, lcb + 512)
                pt_ps = ps_pt.tile([128, 2, 512], F32, tag="pt")
                nc.tensor.matmul(
                    pt_ps[:, 0, off:512],
                    kt[0:64, sb : sb + 128],
                    qt[0:64, lsl],
                    start=True, stop=True, tile_position=(0, 0),
                )
                nc.tensor.matmul(
                    pt_ps[:, 1, off:512],
                    kt[64:128, sb : sb + 128],
                    qt[64:128, lsl],
                    start=True, stop=True, tile_position=(64, 0),
                )
                pt_sb = pts.tile([128, 2, 512], BF16, tag="ptsb")
                nc.scalar.activation(
                    out=pt_sb[:, :, off:512],
                    in_=pt_ps[:, :, off:512],
                    func=EXPF,
                    bias=cdelta[:, js : js + 1],
                    scale=st,
                )
                if sb >= lcb:  # diagonal tile: mask l < s, both branches
                    nc.gpsimd.tensor_mul(
                        pt_sb[:, :, off : off + 128],
                        pt_sb[:, :, off : off + 128],
                        mask,
                    )
                last = js == njs - 1
                nc.tensor.matmul(
                    oac[:, 0, off:512],
                    ve[:, js, :],
                    pt_sb[:, 0, off:512],
                    start=(js == 0), stop=last,
                )
                nc.tensor.matmul(
                    oac[:, 1, off:512],
                    vn[:, js, :],
                    pt_sb[:, 1, off:512],
                    start=(js == 0), stop=last,
                )

            o_sb = osb.tile([D + 1, 2, 512], BF16, tag="osb")
            nc.vector.tensor_copy(out=o_sb, in_=oac)
            # tr4[:, t4, br, 0:64] = O^T block transposed; [.., 64] = denom.
            # 68-wide so every transpose target lands 4B-aligned in PSUM.
            tr4 = ps_tr.tile([128, 4, 2, 68], BF16, tag="tr4")
            for t4 in range(4):
                csl = slice(128 * t4, 128 * t4 + 128)
                for br in range(2):
                    nc.tensor.transpose(
                        tr4[:, t4, br, 0:64], o_sb[0:64, br, csl],
                        ident[0:64, 0:64],
                    )
                    nc.tensor.transpose(
                        tr4[:, t4, br, 64:65], o_sb[64:65, br, csl],
                        ident[64:65, 64:65],
                    )
            rec4 = sml.tile([128, 4, 2], F32, tag="sml")
            nc.vector.reciprocal(rec4, tr4[:, :, :, 64])
            se4 = sml.tile([128, 4, 2], F32, tag="sml")
            nc.vector.tensor_mul(se4, rec4, mtb[:, 4 * lc : 4 * lc + 4, :])
            obuf4 = ob.tile([128, 4, D], F32, tag="ob")
            tmp4 = ob.tile([128, 4, D], F32, tag="tmp")
            nc.vector.tensor_mul(
                obuf4, tr4[:, :, 0, 0:64],
                se4[:, :, 0:1].broadcast_to((128, 4, D)),
            )
            nc.vector.tensor_mul(
                tmp4, tr4[:, :, 1, 0:64],
                se4[:, :, 1:2].broadcast_to((128, 4, D)),
            )
            nc.vector.tensor_add(obuf4, obuf4, tmp4)
            nc.sync.dma_start(
                out=out_p[bh, 512 * lc : 512 * lc + 512, :].rearrange(
                    "(t p) d -> p t d", p=128
                ),
                in_=obuf4,
            )


def _host_in_maps(queries, keys, values, keys_null, values_null, tau, delta,
                  soft_mask):
    ident = np.eye(128, dtype=BF16NP)
    tri = np.triu(np.ones((128, 128), dtype=BF16NP))
    mask = np.ascontiguousarray(
        np.broadcast_to(tri[:, None, :], (128, 2, 128))
    )

    in_maps = []
    for c in range(NCORES):
        b, h0 = c // 4, HPC * (c % 4)
        qt = np.empty((HPC, 128, L), BF16NP)
        kt = np.empty((HPC, 128, S), BF16NP)
        v = np.empty((HPC, S, D + 1), BF16NP)
        vn = np.empty((HPC, S, D + 1), BF16NP)
        for bh in range(HPC):
            h = h0 + bh
            qT = queries[b, :, h, :].T.astype(BF16NP)  # [E, L]
            qt[bh, 0:64] = qT
            qt[bh, 64:128] = qT
            kt[bh, 0:64] = keys[b, :, h, :].T
            kt[bh, 64:128] = keys_null[b, :, h, :].T
            v[bh, :, 0:D] = values[b, :, h, :]
            v[bh, :, D] = 1.0
            vn[bh, :, 0:D] = values_null[b, :, h, :]
            vn[bh, :, D] = 1.0
        m_t = soft_mask[b].reshape(NT, 128).T  # [128, NT]
        mtb = np.empty((128, NT, 2), np.float32)
        mtb[:, :, 0] = m_t
        mtb[:, :, 1] = 1.0 - m_t
        in_maps.append(
            dict(
                qt=qt, kt=kt, v=v, vn=vn,
                st=np.full((128, 1), tau[b, 0] / 8.0, np.float32),
                cdelta=np.ascontiguousarray((delta[b] / 8.0).reshape(NT, 128).T),
                mtb=mtb,
                ident=ident,
                mask=mask,
            )
        )
    return in_maps


def kernel(queries, keys, values, keys_null, values_null, tau, delta, soft_mask):
    global _NC, LAST_EXEC_NS
    queries = np.asarray(queries, dtype=np.float32)
    keys = np.asarray(keys, dtype=np.float32)
    values = np.asarray(values, dtype=np.float32)
    keys_null = np.asarray(keys_null, dtype=np.float32)
    values_null = np.asarray(values_null, dtype=np.float32)
    tau = np.asarray(tau, dtype=np.float32)
    delta = np.asarray(delta, dtype=np.float32)
    soft_mask = np.asarray(soft_mask, dtype=np.float32)

    if _NC is None:
        _NC = _build()

    in_maps = _host_in_maps(
        queries, keys, values, keys_null, values_null, tau, delta, soft_mask
    )
    res = run_bass_kernel_spmd(
        _NC, in_maps, core_ids=list(range(NCORES)), trace=TRACE
    )
    LAST_EXEC_NS = res.exec_time_ns

    out = np.empty((B, L, H, D), np.float32)
    for c in range(NCORES):
        b, h0 = c // 4, HPC * (c % 4)
        out[b, :, h0 : h0 + HPC, :] = res.results[c]["out"].transpose(1, 0, 2)
    return out


# revision 8
# speedup vs baseline: 1.1690x; 1.1690x over previous
"""DSAttention TRN2 Bass kernel.

Reference (per batch b, head h, branch):
    z[l,s] = (q[l]·k[s]) * tau[b]/8 + delta[b,s]/8        (causal: s <= l)
    A = softmax_s(z);  O = A @ V
    out = m*O_edit + (1-m)*O_null,  m = soft_mask[b,l]

Sharding: B*H = 16 (b,h) slices -> 8 cores x 2 heads. Same SPMD program on
every core; core c gets b = c//4, heads 2*(c%4), 2*(c%4)+1.

Per-core algorithm (transposed-score flash attention), v2 = bf16 + merged
tiles to unload the DVE (modeled baseline bottleneck ~92us busy):
  - Host pre-packs per head in bf16: qt = [Q^T; Q^T] (dup) and
    kt = [K_e^T; K_n^T] [128, L] so the two branches' QK^T matmuls run
    row-packed (tile_position (0,0)/(64,0)) concurrently on the PE array,
    and V with a ones column appended ([S, 65]) in natural layout.
  - scores^T tile [s:128, l:<=512] per (S-tile, L-chunk), trimmed to the
    causal region; exp on ACT with fused scale=tau/8 and bias=delta_s/8
    (per-partition APs), both branches in one ACTIVATE, bf16 out; diagonal
    128x128 block masked post-exp with ONE DVE multiply over the
    [128, 2, 128] branch-merged region (bf16 2x mode).
  - O^T accumulates AV matmuls (bf16) into a single merged PSUM tile
    [65, 2, 512] (row 64 of each branch half = softmax denominator via the
    ones column); ONE PSUM->SBUF copy per L-chunk.
  - epilogue per 128-l block: PE-transposes both branches into one PSUM
    bank [128, 2, 65]; DVE: reciprocal of the two denominators in one op,
    one multiply with the host-interleaved (m, 1-m) tile, then
    ts_mul + scalar_tensor_tensor blend -> [128, 64] f32 out.
  - PSUM banks: pt 2x2 + oac 2 + tr 2x1 = 8 (fully used).

REPEAT > 1 wraps the whole per-core program in a hardware For_i loop; used
by the timing harness to measure per-iteration HW time from wall-clock
deltas (transfers cancel).
"""

import contextlib

import ml_dtypes
import numpy as np

import concourse.bass as bass
import concourse.tile as tile
from concourse import bacc, mybir
from concourse.bass_utils import run_bass_kernel_spmd

B, L, S, H, E, D = 2, 2048, 2048, 8, 64, 64
NCORES = 8
HPC = 2            # heads per core
NT = 16            # 128-row tiles in 2048
LCH = 4            # 512-wide L chunks
F32 = mybir.dt.float32
BF16 = mybir.dt.bfloat16
EXPF = mybir.ActivationFunctionType.Exp
MUL = mybir.AluOpType.mult
ADD = mybir.AluOpType.add
BF16NP = ml_dtypes.bfloat16

TRACE = False
LAST_EXEC_NS = None
PTS_BUFS = 8
OSB_BUFS = 4
OB_BUFS = 4
REPEAT = 1

_NC = None


def _build():
    nc = bacc.Bacc("TRN2")
    qt_p = nc.declare_dram_parameter("qt", [HPC, 128, L], BF16, isOutput=False)
    kt_p = nc.declare_dram_parameter("kt", [HPC, 128, S], BF16, isOutput=False)
    v_p = nc.declare_dram_parameter("v", [HPC, S, D + 1], BF16, isOutput=False)
    vn_p = nc.declare_dram_parameter("vn", [HPC, S, D + 1], BF16, isOutput=False)
    st_p = nc.declare_dram_parameter("st", [128, 1], F32, isOutput=False)
    cd_p = nc.declare_dram_parameter("cdelta", [128, NT], F32, isOutput=False)
    mtb_p = nc.declare_dram_parameter("mtb", [128, NT, 2], F32, isOutput=False)
    id_p = nc.declare_dram_parameter("ident", [128, 128], BF16, isOutput=False)
    mk_p = nc.declare_dram_parameter("mask", [128, 2, 128], BF16, isOutput=False)
    out_p = nc.declare_dram_parameter("out", [HPC, L, D], F32, isOutput=True)
    params = (qt_p, kt_p, v_p, vn_p, st_p, cd_p, mtb_p, id_p, mk_p, out_p)

    with tile.TileContext(nc) as tc:
        with (
            tc.tile_pool(name="const", bufs=1) as const,
            tc.tile_pool(name="big", bufs=2) as big,
            tc.tile_pool(name="pts", bufs=PTS_BUFS) as pts,
            tc.tile_pool(name="osb", bufs=OSB_BUFS) as osb,
            tc.tile_pool(name="sml", bufs=8) as sml,
            tc.tile_pool(name="ob", bufs=OB_BUFS) as ob,
            tc.tile_pool(name="ps_pt", bufs=2, space="PSUM") as ps_pt,
            tc.tile_pool(name="ps_oac", bufs=1, space="PSUM") as ps_oac,
            tc.tile_pool(name="ps_tr", bufs=2, space="PSUM") as ps_tr,
        ):
            pools = (const, big, pts, osb, sml, ob, ps_pt, ps_oac, ps_tr)
            rep = (
                tc.For_i(0, REPEAT, 1)
                if REPEAT > 1
                else contextlib.nullcontext()
            )
            with rep:
                _body(nc, pools, params)
    if not nc.is_finalized():
        nc.finalize()
    return nc


def _body(nc, pools, params):
    const, big, pts, osb, sml, ob, ps_pt, ps_oac, ps_tr = pools
    qt_p, kt_p, v_p, vn_p, st_p, cd_p, mtb_p, id_p, mk_p, out_p = params

    ident = const.tile([128, 128], BF16, tag="ident")
    nc.sync.dma_start(out=ident, in_=id_p[:])
    mask = const.tile([128, 2, 128], BF16, tag="mask")
    nc.sync.dma_start(out=mask, in_=mk_p[:])
    cdelta = const.tile([128, NT], F32, tag="cdelta")
    nc.sync.dma_start(out=cdelta, in_=cd_p[:])
    mtb = const.tile([128, NT, 2], F32, tag="mtb")
    nc.sync.dma_start(out=mtb, in_=mtb_p[:])
    st = const.tile([128, 1], F32, tag="st")
    nc.sync.dma_start(out=st, in_=st_p[:])

    for bh in range(HPC):
        qt = big.tile([128, L], BF16, tag="qt")
        kt = big.tile([128, S], BF16, tag="kt")
        nc.sync.dma_start(out=qt, in_=qt_p[bh])
        nc.sync.dma_start(out=kt, in_=kt_p[bh])
        ve = big.tile([128, NT, D + 1], BF16, tag="ve")
        nc.sync.dma_start(
            out=ve, in_=v_p[bh].rearrange("(t p) d -> p t d", p=128)
        )
        vn = big.tile([128, NT, D + 1], BF16, tag="vn")
        nc.sync.dma_start(
            out=vn, in_=vn_p[bh].rearrange("(t p) d -> p t d", p=128)
        )
        for lc in range(LCH):
            lcb = 512 * lc
            oac = ps_oac.tile([D + 1, 2, 512], F32, tag="oac")
            njs = 4 * lc + 4
            for js in range(njs):
                off = max(0, 128 * js - lcb)
                sb = 128 * js
                lsl = slice(lcb + off, lcb + 512)
                pt_ps = ps_pt.tile([128, 2, 512], F32, tag="pt")
                nc.tensor.matmul(
                    pt_ps[:, 0, off:512],
                    kt[0:64, sb : sb + 128],
                    qt[0:64, lsl],
                    start=True, stop=True, tile_position=(0, 0),
                )
                nc.tensor.matmul(
                    pt_ps[:, 1, off:512],
                    kt[64:128, sb : sb + 128],
                    qt[64:128, lsl],
                    start=True, stop=True, tile_position=(64, 0),
                )
                pt_sb = pts.tile([128, 2, 512], BF16, tag="ptsb")
                nc.scalar.activation(
                    out=pt_sb[:, :, off:512],
                    in_=pt_ps[:, :, off:512],
                    func=EXPF,
                    bias=cdelta[:, js : js + 1],
                    scale=st,
                )
                if sb >= lcb:  # diagonal tile: mask l < s, both branches
                    nc.gpsimd.tensor_mul(
                        pt_sb[:, :, off : off + 128],
                        pt_sb[:, :, off : off + 128],
                        mask,
                    )
                last = js == njs - 1
                nc.tensor.matmul(
                    oac[:, 0, off:512],
                    ve[:, js, :],
                    pt_sb[:, 0, off:512],
                    start=(js == 0), stop=last,
                )
                nc.tensor.matmul(
                    oac[:, 1, off:512],
                    vn[:, js, :],
                    pt_sb[:, 1, off:512],
                    start=(js == 0), stop=last,
                )

            o_sb = osb.tile([D + 1, 2, 512], BF16, tag="osb")
            nc.vector.tensor_copy(out=o_sb, in_=oac)
            for t4 in range(4):
                lt = 4 * lc + t4
                csl = slice(128 * t4, 128 * t4 + 128)
                tr = ps_tr.tile([128, 2, D + 2], BF16, tag="tr")
                nc.tensor.transpose(
                    tr[:, 0, 0:65], o_sb[:, 0, csl], ident[0:65, 0:65]
                )
                nc.tensor.transpose(
                    tr[:, 1, 0:65], o_sb[:, 1, csl], ident[0:65, 0:65]
                )
                rec = sml.tile([128, 2], F32, tag="sml")
                nc.vector.reciprocal(rec, tr[:, :, 64:65])
                sesn = sml.tile([128, 2], F32, tag="sml")
                nc.vector.tensor_mul(sesn, rec, mtb[:, lt, :])
                obuf = ob.tile([128, D], F32, tag="ob")
                nc.vector.tensor_scalar_mul(obuf, tr[:, 0, 0:64], sesn[:, 0:1])
                nc.vector.scalar_tensor_tensor(
                    out=obuf, in0=tr[:, 1, 0:64], scalar=sesn[:, 1:2],
                    in1=obuf, op0=MUL, op1=ADD,
                )
                nc.sync.dma_start(
                    out=out_p[bh, 128 * lt : 128 * lt + 128, :],
                    in_=obuf,
                )


def _host_in_maps(queries, keys, values, keys_null, values_null, tau, delta,
                  soft_mask):
    ident = np.eye(128, dtype=BF16NP)
    tri = np.triu(np.ones((128, 128), dtype=BF16NP))
    mask = np.ascontiguousarray(
        np.broadcast_to(tri[:, None, :], (128, 2, 128))
    )

    in_maps = []
    for c in range(NCORES):
        b, h0 = c // 4, HPC * (c % 4)
        qt = np.empty((HPC, 128, L), BF16NP)
        kt = np.empty((HPC, 128, S), BF16NP)
        v = np.empty((HPC, S, D + 1), BF16NP)
        vn = np.empty((HPC, S, D + 1), BF16NP)
        for bh in range(HPC):
            h = h0 + bh
            qT = queries[b, :, h, :].T.astype(BF16NP)  # [E, L]
            qt[bh, 0:64] = qT
            qt[bh, 64:128] = qT
            kt[bh, 0:64] = keys[b, :, h, :].T
            kt[bh, 64:128] = keys_null[b, :, h, :].T
            v[bh, :, 0:D] = values[b, :, h, :]
            v[bh, :, D] = 1.0
            vn[bh, :, 0:D] = values_null[b, :, h, :]
            vn[bh, :, D] = 1.0
        m_t = soft_mask[b].reshape(NT, 128).T  # [128, NT]
        mtb = np.empty((128, NT, 2), np.float32)
        mtb[:, :, 0] = m_t
        mtb[:, :, 1] = 1.0 - m_t
        in_maps.append(
            dict(
                qt=qt, kt=kt, v=v, vn=vn,
                st=np.full((128, 1), tau[b, 0] / 8.0, np.float32),
                cdelta=np.ascontiguousarray((delta[b] / 8.0).reshape(NT, 128).T),
                mtb=mtb,
                ident=ident,
                mask=mask,
            )
        )
    return in_maps


def kernel(queries, keys, values, keys_null, values_null, tau, delta, soft_mask):
    global _NC, LAST_EXEC_NS
    queries = np.asarray(queries, dtype=np.float32)
    keys = np.asarray(keys, dtype=np.float32)
    values = np.asarray(values, dtype=np.float32)
    keys_null = np.asarray(keys_null, dtype=np.float32)
    values_null = np.asarray(values_null, dtype=np.float32)
    tau = np.asarray(tau, dtype=np.float32)
    delta = np.asarray(delta, dtype=np.float32)
    soft_mask = np.asarray(soft_mask, dtype=np.float32)

    if _NC is None:
        _NC = _build()

    in_maps = _host_in_maps(
        queries, keys, values, keys_null, values_null, tau, delta, soft_mask
    )
    res = run_bass_kernel_spmd(
        _NC, in_maps, core_ids=list(range(NCORES)), trace=TRACE
    )
    LAST_EXEC_NS = res.exec_time_ns

    out = np.empty((B, L, H, D), np.float32)
    for c in range(NCORES):
        b, h0 = c // 4, HPC * (c % 4)
        out[b, :, h0 : h0 + HPC, :] = res.results[c]["out"].transpose(1, 0, 2)
    return out


# revision 10
# speedup vs baseline: 1.2196x; 1.0433x over previous
"""DSAttention TRN2 Bass kernel.

Reference (per batch b, head h, branch):
    z[l,s] = (q[l]·k[s]) * tau[b]/8 + delta[b,s]/8        (causal: s <= l)
    A = softmax_s(z);  O = A @ V
    out = m*O_edit + (1-m)*O_null,  m = soft_mask[b,l]

Sharding: B*H = 16 (b,h) slices -> 8 cores x 2 heads. Same SPMD program on
every core; core c gets b = c//4, heads 2*(c%4), 2*(c%4)+1.

Per-core algorithm (transposed-score flash attention), v2 = bf16 + merged
tiles to unload the DVE (modeled baseline bottleneck ~92us busy):
  - Host pre-packs per head in bf16: qt = [Q^T; Q^T] (dup) and
    kt = [K_e^T; K_n^T] [128, L] so the two branches' QK^T matmuls run
    row-packed (tile_position (0,0)/(64,0)) concurrently on the PE array,
    and V with a ones column appended ([S, 65]) in natural layout.
  - scores^T tile [s:128, l:<=512] per (S-tile, L-chunk), trimmed to the
    causal region; exp on ACT with fused scale=tau/8 and bias=delta_s/8
    (per-partition APs), both branches in one ACTIVATE, bf16 out; diagonal
    128x128 block masked post-exp with ONE DVE multiply over the
    [128, 2, 128] branch-merged region (bf16 2x mode).
  - O^T accumulates AV matmuls (bf16) into a single merged PSUM tile
    [65, 2, 512] (row 64 of each branch half = softmax denominator via the
    ones column); ONE PSUM->SBUF copy per L-chunk.
  - epilogue per 128-l block: PE-transposes both branches into one PSUM
    bank [128, 2, 65]; DVE: reciprocal of the two denominators in one op,
    one multiply with the host-interleaved (m, 1-m) tile, then
    ts_mul + scalar_tensor_tensor blend -> [128, 64] f32 out.
  - PSUM banks: pt 2x2 + oac 2 + tr 2x1 = 8 (fully used).

REPEAT > 1 wraps the whole per-core program in a hardware For_i loop; used
by the timing harness to measure per-iteration HW time from wall-clock
deltas (transfers cancel).
"""

import contextlib

import ml_dtypes
import numpy as np

import concourse.bass as bass
import concourse.tile as tile
from concourse import bacc, mybir
from concourse.bass_utils import run_bass_kernel_spmd

B, L, S, H, E, D = 2, 2048, 2048, 8, 64, 64
NCORES = 8
HPC = 2            # heads per core
NT = 16            # 128-row tiles in 2048
LCH = 4            # 512-wide L chunks
F32 = mybir.dt.float32
BF16 = mybir.dt.bfloat16
EXPF = mybir.ActivationFunctionType.Exp
MUL = mybir.AluOpType.mult
ADD = mybir.AluOpType.add
BF16NP = ml_dtypes.bfloat16

TRACE = False
LAST_EXEC_NS = None
PTS_BUFS = 8
OSB_BUFS = 4
OB_BUFS = 4
REPEAT = 1

_NC = None


def _build():
    nc = bacc.Bacc("TRN2")
    qt_p = nc.declare_dram_parameter("qt", [HPC, 128, L], BF16, isOutput=False)
    kt_p = nc.declare_dram_parameter("kt", [HPC, 128, S], BF16, isOutput=False)
    v_p = nc.declare_dram_parameter("v", [HPC, S, D + 1], BF16, isOutput=False)
    vn_p = nc.declare_dram_parameter("vn", [HPC, S, D + 1], BF16, isOutput=False)
    st_p = nc.declare_dram_parameter("st", [128, 1], F32, isOutput=False)
    cd_p = nc.declare_dram_parameter("cdelta", [128, NT], F32, isOutput=False)
    mtb_p = nc.declare_dram_parameter("mtb", [128, NT, 2], F32, isOutput=False)
    id_p = nc.declare_dram_parameter("ident", [128, 128], BF16, isOutput=False)
    mk_p = nc.declare_dram_parameter("mask", [128, 2, 128], BF16, isOutput=False)
    out_p = nc.declare_dram_parameter("out", [HPC, L, D], F32, isOutput=True)
    params = (qt_p, kt_p, v_p, vn_p, st_p, cd_p, mtb_p, id_p, mk_p, out_p)

    with tile.TileContext(nc) as tc:
        with (
            tc.tile_pool(name="const", bufs=1) as const,
            tc.tile_pool(name="big", bufs=2) as big,
            tc.tile_pool(name="pts", bufs=PTS_BUFS) as pts,
            tc.tile_pool(name="osb", bufs=OSB_BUFS) as osb,
            tc.tile_pool(name="sml", bufs=8) as sml,
            tc.tile_pool(name="ob", bufs=OB_BUFS) as ob,
            tc.tile_pool(name="ps_pt", bufs=2, space="PSUM") as ps_pt,
            tc.tile_pool(name="ps_oac", bufs=1, space="PSUM") as ps_oac,
            tc.tile_pool(name="ps_tr", bufs=2, space="PSUM") as ps_tr,
        ):
            pools = (const, big, pts, osb, sml, ob, ps_pt, ps_oac, ps_tr)
            rep = (
                tc.For_i(0, REPEAT, 1)
                if REPEAT > 1
                else contextlib.nullcontext()
            )
            with rep:
                _body(nc, pools, params)
    if not nc.is_finalized():
        nc.finalize()
    return nc


def _body(nc, pools, params):
    const, big, pts, osb, sml, ob, ps_pt, ps_oac, ps_tr = pools
    qt_p, kt_p, v_p, vn_p, st_p, cd_p, mtb_p, id_p, mk_p, out_p = params

    ident = const.tile([128, 128], BF16, tag="ident")
    nc.sync.dma_start(out=ident, in_=id_p[:])
    mask = const.tile([128, 2, 128], BF16, tag="mask")
    nc.sync.dma_start(out=mask, in_=mk_p[:])
    cdelta = const.tile([128, NT], F32, tag="cdelta")
    nc.sync.dma_start(out=cdelta, in_=cd_p[:])
    mtb = const.tile([128, NT, 2], F32, tag="mtb")
    nc.sync.dma_start(out=mtb, in_=mtb_p[:])
    st = const.tile([128, 1], F32, tag="st")
    nc.sync.dma_start(out=st, in_=st_p[:])

    for bh in range(HPC):
        qt = big.tile([128, L], BF16, tag="qt")
        kt = big.tile([128, S], BF16, tag="kt")
        nc.sync.dma_start(out=qt, in_=qt_p[bh])
        nc.sync.dma_start(out=kt, in_=kt_p[bh])
        ve = big.tile([128, NT, D + 1], BF16, tag="ve")
        nc.sync.dma_start(
            out=ve, in_=v_p[bh].rearrange("(t p) d -> p t d", p=128)
        )
        vn = big.tile([128, NT, D + 1], BF16, tag="vn")
        nc.sync.dma_start(
            out=vn, in_=vn_p[bh].rearrange("(t p) d -> p t d", p=128)
        )
        for lc in range(LCH):
            lcb = 512 * lc
            oac_e = ps_oac.tile([D + 1, 512], F32, tag="oe")
            oac_n = ps_oac.tile([D + 1, 512], F32, tag="on")
            njs = 4 * lc + 4
            for js in range(njs):
                off = max(0, 128 * js - lcb)
                sb = 128 * js
                lsl = slice(lcb + off, lcb + 512)
                pt_ps = ps_pt.tile([128, 2, 512], F32, tag="pt")
                nc.tensor.matmul(
                    pt_ps[:, 0, off:512],
                    kt[0:64, sb : sb + 128],
                    qt[0:64, lsl],
                    start=True, stop=True, tile_position=(0, 0),
                )
                nc.tensor.matmul(
                    pt_ps[:, 1, off:512],
                    kt[64:128, sb : sb + 128],
                    qt[64:128, lsl],
                    start=True, stop=True, tile_position=(64, 0),
                )
                pt_sb = pts.tile([128, 2, 512], BF16, tag="ptsb")
                nc.scalar.activation(
                    out=pt_sb[:, :, off:512],
                    in_=pt_ps[:, :, off:512],
                    func=EXPF,
                )
                if sb >= lcb:  # diagonal tile: mask l < s, both branches
                    nc.gpsimd.tensor_mul(
                        pt_sb[:, :, off : off + 128],
                        pt_sb[:, :, off : off + 128],
                        mask,
                    )
                last = js == njs - 1
                nc.tensor.matmul(
                    oac_e[:, off:512],
                    ve[:, js, :],
                    pt_sb[:, 0, off:512],
                    start=(js == 0), stop=last,
                )
                nc.tensor.matmul(
                    oac_n[:, off:512],
                    vn[:, js, :],
                    pt_sb[:, 1, off:512],
                    start=(js == 0), stop=last,
                )

            o_sb = osb.tile([D + 1, 2, 512], BF16, tag="osb")
            nc.vector.tensor_copy(out=o_sb[:, 0, :], in_=oac_e)
            nc.vector.tensor_copy(out=o_sb[:, 1, :], in_=oac_n)
            for t4 in range(4):
                lt = 4 * lc + t4
                csl = slice(128 * t4, 128 * t4 + 128)
                tr = ps_tr.tile([128, 2, D + 2], BF16, tag="tr")
                nc.tensor.transpose(
                    tr[:, 0, 0:65], o_sb[:, 0, csl], ident[0:65, 0:65]
                )
                nc.tensor.transpose(
                    tr[:, 1, 0:65], o_sb[:, 1, csl], ident[0:65, 0:65]
                )
                rec = sml.tile([128, 2], F32, tag="sml")
                nc.vector.reciprocal(rec, tr[:, :, 64:65])
                sesn = sml.tile([128, 2], F32, tag="sml")
                nc.vector.tensor_mul(sesn, rec, mtb[:, lt, :])
                obuf = ob.tile([128, D], F32, tag="ob")
                nc.vector.tensor_scalar_mul(obuf, tr[:, 0, 0:64], sesn[:, 0:1])
                nc.vector.scalar_tensor_tensor(
                    out=obuf, in0=tr[:, 1, 0:64], scalar=sesn[:, 1:2],
                    in1=obuf, op0=MUL, op1=ADD,
                )
                nc.sync.dma_start(
                    out=out_p[bh, 128 * lt : 128 * lt + 128, :],
                    in_=obuf,
                )


def _host_in_maps(queries, keys, values, keys_null, values_null, tau, delta,
                  soft_mask):
    ident = np.eye(128, dtype=BF16NP)
    tri = np.triu(np.ones((128, 128), dtype=BF16NP))
    mask = np.ascontiguousarray(
        np.broadcast_to(tri[:, None, :], (128, 2, 128))
    )

    in_maps = []
    for c in range(NCORES):
        b, h0 = c // 4, HPC * (c % 4)
        qt = np.empty((HPC, 128, L), BF16NP)
        kt = np.empty((HPC, 128, S), BF16NP)
        v = np.empty((HPC, S, D + 1), BF16NP)
        vn = np.empty((HPC, S, D + 1), BF16NP)
        for bh in range(HPC):
            h = h0 + bh
            qT = queries[b, :, h, :].T.astype(BF16NP)  # [E, L]
            qt[bh, 0:64] = qT
            qt[bh, 64:128] = qT
            ts = tau[b, 0] / 8.0
            ed = np.exp(delta[b] / 8.0).astype(np.float32)  # [S]
            kt[bh, 0:64] = keys[b, :, h, :].T * ts
            kt[bh, 64:128] = keys_null[b, :, h, :].T * ts
            v[bh, :, 0:D] = values[b, :, h, :] * ed[:, None]
            v[bh, :, D] = ed
            vn[bh, :, 0:D] = values_null[b, :, h, :] * ed[:, None]
            vn[bh, :, D] = ed
        m_t = soft_mask[b].reshape(NT, 128).T  # [128, NT]
        mtb = np.empty((128, NT, 2), np.float32)
        mtb[:, :, 0] = m_t
        mtb[:, :, 1] = 1.0 - m_t
        in_maps.append(
            dict(
                qt=qt, kt=kt, v=v, vn=vn,
                st=np.full((128, 1), tau[b, 0] / 8.0, np.float32),
                cdelta=np.ascontiguousarray((delta[b] / 8.0).reshape(NT, 128).T),
                mtb=mtb,
                ident=ident,
                mask=mask,
            )
        )
    return in_maps


def kernel(queries, keys, values, keys_null, values_null, tau, delta, soft_mask):
    global _NC, LAST_EXEC_NS
    queries = np.asarray(queries, dtype=np.float32)
    keys = np.asarray(keys, dtype=np.float32)
    values = np.asarray(values, dtype=np.float32)
    keys_null = np.asarray(keys_null, dtype=np.float32)
    values_null = np.asarray(values_null, dtype=np.float32)
    tau = np.asarray(tau, dtype=np.float32)
    delta = np.asarray(delta, dtype=np.float32)
    soft_mask = np.asarray(soft_mask, dtype=np.float32)

    if _NC is None:
        _NC = _build()

    in_maps = _host_in_maps(
        queries, keys, values, keys_null, values_null, tau, delta, soft_mask
    )
    res = run_bass_kernel_spmd(
        _NC, in_maps, core_ids=list(range(NCORES)), trace=TRACE
    )
    LAST_EXEC_NS = res.exec_time_ns

    out = np.empty((B, L, H, D), np.float32)
    for c in range(NCORES):
        b, h0 = c // 4, HPC * (c % 4)
        out[b, :, h0 : h0 + HPC, :] = res.results[c]["out"].transpose(1, 0, 2)
    return out
